# revision 19
# baseline (speedup 1.0000x reference)
import sys

sys.path.insert(0, "/opt/trn_rl_repo")

import numpy as np
import ml_dtypes

from concourse import bass, bacc, tile, mybir
from concourse.bass_utils import run_bass_kernel_spmd

B = 8192
NPG = 50
EPG = 100
N = B * NPG
E = B * EPG
F1, F2, F3 = 78, 156, 312
NCORES = 8
GPC = B // NCORES          # 1024 graphs per core
NPC = GPC * NPG            # 51200 nodes per core
PAIRS = GPC // 2           # 512 graph-pairs per core
GRP = 8                    # pairs per DMA group
NGRP = PAIRS // GRP        # 64 DMA groups

BF16 = mybir.dt.bfloat16
F32 = mybir.dt.float32
NP_BF16 = ml_dtypes.bfloat16
RELU = mybir.ActivationFunctionType.Relu
IDENT = mybir.ActivationFunctionType.Identity
MAXOP = mybir.AluOpType.max
AXX = mybir.AxisListType.X

_CACHE = {}


def _build_at_pairs(edge_index):
    """Host: normalized GCN adjacency, transposed, pair-block-diagonal.

    AT[g, s, d] = dinv[src]*dinv[dst] summed over edges, AT[g, i, i] += dinv^2
    so that (A_hat @ H) == (H^T @ AT)^T per graph, matching the reference
    segment_sum formulation exactly.
    """
    src = np.asarray(edge_index[0], dtype=np.int64)
    dst = np.asarray(edge_index[1], dtype=np.int64)
    deg = np.bincount(dst, minlength=N).astype(np.float32) + 1.0
    dinv = 1.0 / np.sqrt(deg)
    norm = (dinv[src] * dinv[dst]).astype(np.float64)
    g = dst // NPG
    sl = src - g * NPG
    dl = dst - g * NPG
    flat = g * (NPG * NPG) + sl * NPG + dl
    at = np.bincount(flat, weights=norm, minlength=B * NPG * NPG)
    at = at.astype(np.float32).reshape(B, NPG, NPG)
    d2 = (dinv * dinv).reshape(B, NPG)
    ii = np.arange(NPG)
    at[:, ii, ii] += d2
    atp = np.zeros((B // 2, 2 * NPG, 2 * NPG), dtype=np.float32)
    atp[:, :NPG, :NPG] = at[0::2]
    atp[:, NPG:, NPG:] = at[1::2]
    # group for DMA batching: [NCORES, NGRP, 100, GRP*100]
    atp = atp.astype(NP_BF16).reshape(NCORES, NGRP, GRP, 100, 100)
    atp = np.ascontiguousarray(atp.transpose(0, 1, 3, 2, 4)).reshape(
        NCORES, NGRP, 100, GRP * 100
    )
    return atp


def _tile_p1(x, W1):
    """Host-fold the first GCN linear (no activation precedes it):
    p1 = x @ W1, tiled per core as [NGRP, 100, GRP*78] node-major pair
    groups so it can serve directly as the agg1 stationary."""
    p1 = np.asarray(x, dtype=np.float32) @ np.asarray(W1, dtype=np.float32)
    p1 = p1.astype(NP_BF16).reshape(NCORES, NGRP, GRP, 100, F1)
    return np.ascontiguousarray(p1.transpose(0, 1, 3, 2, 4)).reshape(
        NCORES, NGRP, 100, GRP * F1
    )


def _prep_cell(cell):
    cell = np.asarray(cell, dtype=np.float32)
    nrm = np.sqrt((cell * cell).sum(axis=1, keepdims=True))
    cv = cell / np.maximum(nrm, 1e-12)
    cv = cv.reshape(NCORES, GPC, 954)
    cv = np.ascontiguousarray(cv.transpose(0, 2, 1))  # [NCORES, 954, GPC]
    return cv.reshape(NCORES, 9, 106, GPC).astype(NP_BF16)


def _wchunk(w, kc):
    """[K, M] -> [kc, K//kc? ...] -> sbuf layout [kchunk_rows, nchunks, M]."""
    K, M = w.shape
    n = K // kc
    return np.ascontiguousarray(
        w.reshape(n, kc, M).transpose(1, 0, 2)
    )


def _bchunk(b, pc):
    """[F] -> [F//pc, pc]: column c holds chunk c of the bias, fp32."""
    return np.ascontiguousarray(b.reshape(pc, -1).T).astype(np.float32)


def _build_program():
    nc = bacc.Bacc("TRN2", target_bir_lowering=False, debug=False)

    def din(name, shape, dt=BF16):
        return nc.dram_tensor(name, list(shape), dt, kind="ExternalInput").ap()

    x1p = din("x1p", (NGRP, 100, GRP * F1))
    x2p = din("x2p", (NGRP, 100, GRP * F1))
    a1p = din("a1p", (NGRP, 100, GRP * 100))
    a2p = din("a2p", (NGRP, 100, GRP * 100))
    cellc = din("cellc", (9, 106, GPC))

    wc2_d = din("wc2", (F1, F2))
    wc3_d = din("wc3", (78, 2, F3))
    wg1_d = din("wg1", (104, 3, F2))
    wg2_d = din("wg2", (78, 2, 128))
    wr1_d = din("wr1", (106, 9, 512))
    wr2_d = din("wr2", (128, 4, 256))
    wr3_d = din("wr3", (128, 2, 128))
    wf1_d = din("wf1", (128, 3, 256))
    wf2_d = din("wf2", (128, 2, 128))
    wo_d = din("wo", (128, 2))

    bc1_d = din("bc1", (78, 1), F32)
    bc2_d = din("bc2", (78, 2), F32)
    bc3_d = din("bc3", (104, 3), F32)
    bg1_d = din("bg1", (78, 2), F32)
    bg2_d = din("bg2", (128, 1), F32)
    br1_d = din("br1", (128, 4), F32)
    br2_d = din("br2", (128, 2), F32)
    br3_d = din("br3", (128, 1), F32)
    bf1_d = din("bf1", (128, 2), F32)
    bf2_d = din("bf2", (128, 1), F32)
    bo_d = din("bo", (2, 1), F32)

    out_d = nc.dram_tensor("outT", [2, GPC], F32, kind="ExternalOutput").ap()

    with tile.TileContext(nc) as tc:
        from contextlib import ExitStack

        with ExitStack() as ctx:
            cpool = ctx.enter_context(tc.tile_pool(name="consts", bufs=1))

            def load(dram, shape, dt=BF16):
                nm = dram.name.split("_")[0]
                t = cpool.tile(list(shape), dt, tag=nm, name=nm)
                nc.sync.dma_start(t[:], dram[:])
                return t

            wc2 = load(wc2_d, (F1, F2))
            wc3 = load(wc3_d, (78, 2, F3))
            wg1 = load(wg1_d, (104, 3, F2))
            wg2 = load(wg2_d, (78, 2, 128))
            wr1 = load(wr1_d, (106, 9, 512))
            wr2 = load(wr2_d, (128, 4, 256))
            wr3 = load(wr3_d, (128, 2, 128))
            wf1 = load(wf1_d, (128, 3, 256))
            wf2 = load(wf2_d, (128, 2, 128))
            wo = load(wo_d, (128, 2))
            bc1 = load(bc1_d, (78, 1), F32)
            bc2 = load(bc2_d, (78, 2), F32)
            bc3 = load(bc3_d, (104, 3), F32)
            bg1 = load(bg1_d, (78, 2), F32)
            bg2 = load(bg2_d, (128, 1), F32)
            br1 = load(br1_d, (128, 4), F32)
            br2 = load(br2_d, (128, 2), F32)
            br3 = load(br3_d, (128, 1), F32)
            bf1 = load(bf1_d, (128, 2), F32)
            bf2 = load(bf2_d, (128, 1), F32)
            bo = load(bo_d, (2, 1), F32)

            # persistent per-branch outputs
            pooled_raw = [
                [
                    cpool.tile([104, GPC], BF16, tag=f"pr{d}{c}", name=f"pr{d}{c}")
                    for c in range(3)
                ]
                for d in range(2)
            ]
            pooled = [
                [
                    cpool.tile([104, GPC], BF16, tag=f"pool{d}{c}", name=f"pool{d}{c}")
                    for c in range(3)
                ]
                for d in range(2)
            ]
            demb = [
                cpool.tile([128, GPC], BF16, tag=f"demb{d}", name=f"demb{d}")
                for d in range(2)
            ]
            c3T = cpool.tile([128, GPC], BF16, tag="c3T", name="c3T")

            # ---------------- cell branch (runs first: its DMAs prefetch
            # at t=0 and its long accumulation chains warm up the PE) ----
            with tc.tile_pool(name="cellp", bufs=1) as clp, tc.tile_pool(
                name="pscell", bufs=2, space=bass.MemorySpace.PSUM
            ) as cps:
                cell_sb = []
                for k in range(9):
                    t = clp.tile([106, GPC], BF16, tag=f"cell{k}", name=f"cell{k}")
                    nc.sync.dma_start(t[:], cellc[k])
                    cell_sb.append(t)
                c1 = clp.tile([128, 4 * GPC], BF16, tag="c1", name="c1")
                for m in range(4):
                    for n in range(2):
                        ps = cps.tile([128, 512], F32, tag="ps", name="ps")
                        for k in range(9):
                            nc.tensor.matmul(
                                ps[:],
                                wr1[:, k, m * 128 : (m + 1) * 128],
                                cell_sb[k][:, n * 512 : (n + 1) * 512],
                                start=(k == 0),
                                stop=(k == 8),
                            )
                        nc.scalar.activation(
                            c1[:, m * GPC + n * 512 : m * GPC + (n + 1) * 512],
                            ps[:],
                            RELU,
                            bias=br1[:, m : m + 1],
                        )
                c2 = clp.tile([128, 2 * GPC], BF16, tag="c2", name="c2")
                for m in range(2):
                    for n in range(2):
                        ps = cps.tile([128, 512], F32, tag="ps", name="ps")
                        for k in range(4):
                            nc.tensor.matmul(
                                ps[:],
                                wr2[:, k, m * 128 : (m + 1) * 128],
                                c1[:, k * GPC + n * 512 : k * GPC + (n + 1) * 512],
                                start=(k == 0),
                                stop=(k == 3),
                            )
                        nc.scalar.activation(
                            c2[:, m * GPC + n * 512 : m * GPC + (n + 1) * 512],
                            ps[:],
                            RELU,
                            bias=br2[:, m : m + 1],
                        )
                for n in range(2):
                    ps = cps.tile([128, 512], F32, tag="ps", name="ps")
                    for k in range(2):
                        nc.tensor.matmul(
                            ps[:],
                            wr3[:, k, :],
                            c2[:, k * GPC + n * 512 : k * GPC + (n + 1) * 512],
                            start=(k == 0),
                            stop=(k == 1),
                        )
                    nc.scalar.activation(
                        c3T[:, n * 512 : (n + 1) * 512], ps[:], IDENT, bias=br3[:]
                    )

            # ---------------- drug branches ----------------
            # Per group of 8 pairs: FOUR interleaved streams (2 drugs x 2
            # quads of 4 pairs) so the per-stream dependency chain
            # (agg1 -> act1 -> xw2 -> cast -> agg2 -> ...) is covered by
            # ~3 phases of other streams' PE work and the PE never idles
            # (keeps the tensor engine in the high DVFS p-state).
            # L3 uses the relu<->max swap: pool the raw aggregation from
            # PSUM, defer bias+relu to the pooled [104, GPC] tensor.
            with tc.tile_pool(name="io", bufs=6) as iop, tc.tile_pool(
                name="mid", bufs=8
            ) as midp, tc.tile_pool(name="p3p", bufs=16) as p3p, tc.tile_pool(
                name="psb", bufs=2, space=bass.MemorySpace.PSUM
            ) as psum:
                drug_io = ((x1p, a1p), (x2p, a2p))
                for gi in range(NGRP):
                    tiles = []
                    for d, (xp, ap) in enumerate(drug_io):
                        p1t = iop.tile([100, GRP * F1], BF16, tag="p1t", name="p1t")
                        nc.sync.dma_start(p1t[:], xp[gi])
                        at = iop.tile([100, GRP * 100], BF16, tag="at", name="at")
                        nc.sync.dma_start(at[:], ap[gi])
                        tiles.append((p1t, at))

                    def make_quad(d, q):
                        p1t, at = tiles[d]
                        base = q * 4
                        st = {}

                        def pcols(j):
                            o = (base + j) * 100
                            return slice(o, o + 100)

                        def p2_agg1():
                            ph1 = psum.tile([78, 400], F32, tag="ph", name="ph1")
                            for j in range(4):
                                o = (base + j) * F1
                                nc.tensor.matmul(
                                    ph1[:, 100 * j : 100 * j + 100],
                                    p1t[:, o : o + F1],
                                    at[:, pcols(j)],
                                    start=True,
                                    stop=True,
                                )
                            h1q = midp.tile([78, 400], BF16, tag="h1q", name="h1q")
                            nc.scalar.activation(h1q[:], ph1[:], RELU, bias=bc1[:])
                            st["h1q"] = h1q

                        def p3_xw2():
                            h1q = st["h1q"]
                            pp2 = [
                                psum.tile([100, 312], F32, tag="pp", name="pp2", bufs=4)
                                for _ in range(2)
                            ]
                            for j in range(4):
                                nc.tensor.matmul(
                                    pp2[j // 2][:, 156 * (j % 2) : 156 * (j % 2) + 156],
                                    h1q[:, 100 * j : 100 * j + 100],
                                    wc2[:],
                                    start=True,
                                    stop=True,
                                )
                            p2q = [
                                midp.tile([100, 312], BF16, tag="p2q", name="p2q")
                                for _ in range(2)
                            ]
                            nc.scalar.activation(
                                p2q[0][:, 0:156], pp2[0][:, 0:156], IDENT
                            )
                            nc.scalar.activation(
                                p2q[0][:, 156:312], pp2[0][:, 156:312], IDENT
                            )
                            nc.vector.tensor_copy(
                                p2q[1][:, 0:156], pp2[1][:, 0:156]
                            )
                            nc.vector.tensor_copy(
                                p2q[1][:, 156:312], pp2[1][:, 156:312]
                            )
                            st["p2q"] = p2q

                        def p4_agg2():
                            p2q = st["p2q"]
                            ph2 = [
                                psum.tile([78, 400], F32, tag="ph", name="ph2")
                                for _ in range(2)
                            ]
                            for c in range(2):
                                for j in range(4):
                                    o = 156 * (j % 2) + 78 * c
                                    nc.tensor.matmul(
                                        ph2[c][:, 100 * j : 100 * j + 100],
                                        p2q[j // 2][:, o : o + 78],
                                        at[:, pcols(j)],
                                        start=True,
                                        stop=True,
                                    )
                            h2q = midp.tile([78, 800], BF16, tag="h2q", name="h2q")
                            for c in range(2):
                                nc.scalar.activation(
                                    h2q[:, 400 * c : 400 * c + 400],
                                    ph2[c][:],
                                    RELU,
                                    bias=bc2[:, c : c + 1],
                                )
                            st["h2q"] = h2q

                        def p5_xw3():
                            h2q = st["h2q"]
                            p3l = []
                            for j in range(4):
                                pp3 = psum.tile(
                                    [100, 312], F32, tag="pp", name="pp3", bufs=4
                                )
                                nc.tensor.matmul(
                                    pp3[:],
                                    h2q[:, 100 * j : 100 * j + 100],
                                    wc3[:, 0, :],
                                    start=True,
                                    stop=False,
                                )
                                nc.tensor.matmul(
                                    pp3[:],
                                    h2q[:, 400 + 100 * j : 400 + 100 * j + 100],
                                    wc3[:, 1, :],
                                    start=False,
                                    stop=True,
                                )
                                p3 = p3p.tile([100, 312], BF16, tag="p3", name="p3")
                                # chunk-0 half first: agg3's first chunk only
                                # needs cols 0:156 of every pair's p3
                                if j % 2 == 0:
                                    nc.scalar.activation(
                                        p3[:, 0:156], pp3[:, 0:156], IDENT
                                    )
                                    nc.scalar.activation(
                                        p3[:, 156:312], pp3[:, 156:312], IDENT
                                    )
                                else:
                                    nc.vector.tensor_copy(
                                        p3[:, 0:156], pp3[:, 0:156]
                                    )
                                    nc.vector.tensor_copy(
                                        p3[:, 156:312], pp3[:, 156:312]
                                    )
                                p3l.append(p3)
                            st["p3l"] = p3l

                        def p6_agg3():
                            p3l = st["p3l"]
                            goff = 2 * (gi * GRP + base)
                            for c in range(3):
                                ph3 = psum.tile(
                                    [104, 8, 50], F32, tag="ph3", name="ph3"
                                )
                                for j in range(4):
                                    nc.tensor.matmul(
                                        ph3[:, 2 * j : 2 * j + 2, :],
                                        p3l[j][:, 104 * c : 104 * c + 104],
                                        at[:, pcols(j)],
                                        start=True,
                                        stop=True,
                                    )
                                nc.vector.tensor_reduce(
                                    pooled_raw[d][c][:, goff : goff + 8],
                                    ph3[:],
                                    AXX,
                                    MAXOP,
                                )

                        return (p2_agg1, p3_xw2, p4_agg2, p5_xw3, p6_agg3)

                    streams = [make_quad(d, q) for d in range(2) for q in range(2)]
                    for phase_fns in zip(*streams):
                        for fn in phase_fns:
                            fn()

            # ---------------- drug FC heads ----------------
            with tc.tile_pool(name="fc", bufs=1) as pool, tc.tile_pool(
                name="psfc", bufs=2, space=bass.MemorySpace.PSUM
            ) as psum:
                # deferred bias+relu of the max-pooled GCN outputs
                for d in range(2):
                    for c in range(3):
                        nc.scalar.activation(
                            pooled[d][c][:],
                            pooled_raw[d][c][:],
                            RELU,
                            bias=bc3[:, c : c + 1],
                        )
                for d in range(2):
                    gfc = pool.tile([78, 2 * GPC], BF16, tag=f"gfc{d}", name=f"gfc{d}")
                    for m in range(2):
                        for n in range(2):
                            ps = psum.tile([78, 512], F32, tag="ps", name="ps")
                            for k in range(3):
                                nc.tensor.matmul(
                                    ps[:],
                                    wg1[:, k, m * 78 : (m + 1) * 78],
                                    pooled[d][k][:, n * 512 : (n + 1) * 512],
                                    start=(k == 0),
                                    stop=(k == 2),
                                )
                            nc.scalar.activation(
                                gfc[:, m * GPC + n * 512 : m * GPC + (n + 1) * 512],
                                ps[:],
                                RELU,
                                bias=bg1[:, m : m + 1],
                            )
                    for n in range(2):
                        ps = psum.tile([128, 512], F32, tag="ps", name="ps")
                        for k in range(2):
                            nc.tensor.matmul(
                                ps[:],
                                wg2[:, k, :],
                                gfc[:, k * GPC + n * 512 : k * GPC + (n + 1) * 512],
                                start=(k == 0),
                                stop=(k == 1),
                            )
                        nc.scalar.activation(
                            demb[d][:, n * 512 : (n + 1) * 512],
                            ps[:],
                            IDENT,
                            bias=bg2[:],
                        )

                # ---------------- head ----------------
                xcs = [demb[0], demb[1], c3T]
                hf1 = pool.tile([128, 2 * GPC], BF16, tag="hf1", name="hf1")
                for m in range(2):
                    for n in range(2):
                        ps = psum.tile([128, 512], F32, tag="ps", name="ps")
                        for k in range(3):
                            nc.tensor.matmul(
                                ps[:],
                                wf1[:, k, m * 128 : (m + 1) * 128],
                                xcs[k][:, n * 512 : (n + 1) * 512],
                                start=(k == 0),
                                stop=(k == 2),
                            )
                        nc.scalar.activation(
                            hf1[:, m * GPC + n * 512 : m * GPC + (n + 1) * 512],
                            ps[:],
                            RELU,
                            bias=bf1[:, m : m + 1],
                        )
                hf2 = pool.tile([128, GPC], BF16, tag="hf2", name="hf2")
                for n in range(2):
                    ps = psum.tile([128, 512], F32, tag="ps", name="ps")
                    for k in range(2):
                        nc.tensor.matmul(
                            ps[:],
                            wf2[:, k, :],
                            hf1[:, k * GPC + n * 512 : k * GPC + (n + 1) * 512],
                            start=(k == 0),
                            stop=(k == 1),
                        )
                    nc.scalar.activation(
                        hf2[:, n * 512 : (n + 1) * 512], ps[:], RELU, bias=bf2[:]
                    )
                osb = pool.tile([2, GPC], F32, tag="osb", name="osb")
                for n in range(2):
                    ps = psum.tile([2, 512], F32, tag="ps", name="ps")
                    nc.tensor.matmul(
                        ps[:],
                        wo[:],
                        hf2[:, n * 512 : (n + 1) * 512],
                        start=True,
                        stop=True,
                    )
                    nc.scalar.activation(
                        osb[:, n * 512 : (n + 1) * 512], ps[:], IDENT, bias=bo[:]
                    )
                nc.sync.dma_start(out_d[:], osb[:])

    nc.compile()
    return nc


def kernel(x1, edge_index1, batch1, x2, edge_index2, batch2, cell,
           Wc1, bc1, Wc2, bc2, Wc3, bc3, Wg1, bg1, Wg2, bg2,
           Wr1, br1, Wr2, br2, Wr3, br3, Wf1, bf1, Wf2, bf2, Wo, bo):
    if "nc" not in _CACHE:
        _CACHE["nc"] = _build_program()
    nc = _CACHE["nc"]

    x1p = _tile_p1(x1, Wc1)
    x2p = _tile_p1(x2, Wc1)
    a1p = _build_at_pairs(edge_index1)
    a2p = _build_at_pairs(edge_index2)
    cellc = _prep_cell(cell)

    bf = lambda a: np.asarray(a, dtype=np.float32).astype(NP_BF16)
    f32 = lambda a: np.asarray(a, dtype=np.float32)
    shared = dict(
        wc2=bf(Wc2),
        wc3=bf(_wchunk(np.asarray(Wc3, np.float32), 78)),
        wg1=bf(_wchunk(np.asarray(Wg1, np.float32), 104)),
        wg2=bf(_wchunk(np.asarray(Wg2, np.float32), 78)),
        wr1=bf(_wchunk(np.asarray(Wr1, np.float32), 106)),
        wr2=bf(_wchunk(np.asarray(Wr2, np.float32), 128)),
        wr3=bf(_wchunk(np.asarray(Wr3, np.float32), 128)),
        wf1=bf(_wchunk(np.asarray(Wf1, np.float32), 128)),
        wf2=bf(_wchunk(np.asarray(Wf2, np.float32), 128)),
        wo=bf(Wo),
        bc1=f32(bc1).reshape(78, 1),
        bc2=_bchunk(f32(bc2), 2),
        bc3=_bchunk(f32(bc3), 3),
        bg1=_bchunk(f32(bg1), 2),
        bg2=f32(bg2).reshape(128, 1),
        br1=_bchunk(f32(br1), 4),
        br2=_bchunk(f32(br2), 2),
        br3=f32(br3).reshape(128, 1),
        bf1=_bchunk(f32(bf1), 2),
        bf2=f32(bf2).reshape(128, 1),
        bo=f32(bo).reshape(2, 1),
    )

    in_maps = []
    for c in range(NCORES):
        m = dict(shared)
        m["x1p"] = x1p[c]
        m["x2p"] = x2p[c]
        m["a1p"] = a1p[c]
        m["a2p"] = a2p[c]
        m["cellc"] = cellc[c]
        in_maps.append(m)

    res = run_bass_kernel_spmd(nc, in_maps, list(range(NCORES)))
    _CACHE["last_result"] = res
    out = np.concatenate(
        [np.asarray(res.results[c]["outT"], np.float32).T for c in range(NCORES)],
        axis=0,
    )
    return out


# revision 26
# speedup vs baseline: 1.0939x; 1.0939x over previous
import sys

sys.path.insert(0, "/opt/trn_rl_repo")

import numpy as np
import ml_dtypes

from concourse import bass, bacc, tile, mybir
from concourse.bass_utils import run_bass_kernel_spmd

B = 8192
NPG = 50
EPG = 100
N = B * NPG
E = B * EPG
F1, F2, F3 = 78, 156, 312
NCORES = 8
GPC = B // NCORES          # 1024 graphs per core
NPC = GPC * NPG            # 51200 nodes per core
PAIRS = GPC // 2           # 512 graph-pairs per core
GRP = 8                    # pairs per DMA group
NGRP = PAIRS // GRP        # 64 DMA groups

BF16 = mybir.dt.bfloat16
F32 = mybir.dt.float32
NP_BF16 = ml_dtypes.bfloat16
RELU = mybir.ActivationFunctionType.Relu
IDENT = mybir.ActivationFunctionType.Identity
MAXOP = mybir.AluOpType.max
AXX = mybir.AxisListType.X

_CACHE = {}


def _build_at_pairs(edge_index):
    """Host: normalized GCN adjacency, transposed, pair-block-diagonal.

    AT[g, s, d] = dinv[src]*dinv[dst] summed over edges, AT[g, i, i] += dinv^2
    so that (A_hat @ H) == (H^T @ AT)^T per graph, matching the reference
    segment_sum formulation exactly.
    """
    src = np.asarray(edge_index[0], dtype=np.int64)
    dst = np.asarray(edge_index[1], dtype=np.int64)
    deg = np.bincount(dst, minlength=N).astype(np.float32) + 1.0
    dinv = 1.0 / np.sqrt(deg)
    norm = (dinv[src] * dinv[dst]).astype(np.float64)
    g = dst // NPG
    sl = src - g * NPG
    dl = dst - g * NPG
    flat = g * (NPG * NPG) + sl * NPG + dl
    at = np.bincount(flat, weights=norm, minlength=B * NPG * NPG)
    at = at.astype(np.float32).reshape(B, NPG, NPG)
    d2 = (dinv * dinv).reshape(B, NPG)
    ii = np.arange(NPG)
    at[:, ii, ii] += d2
    atp = np.zeros((B // 2, 2 * NPG, 2 * NPG), dtype=np.float32)
    atp[:, :NPG, :NPG] = at[0::2]
    atp[:, NPG:, NPG:] = at[1::2]
    # group for DMA batching: [NCORES, NGRP, 100, GRP*100]
    atp = atp.astype(NP_BF16).reshape(NCORES, NGRP, GRP, 100, 100)
    atp = np.ascontiguousarray(atp.transpose(0, 1, 3, 2, 4)).reshape(
        NCORES, NGRP, 100, GRP * 100
    )
    return atp


def _tile_p1(x, W1):
    """Host-fold the first GCN linear (no activation precedes it):
    p1 = x @ W1, tiled per core as [NGRP, 100, GRP*78] node-major pair
    groups so it can serve directly as the agg1 stationary."""
    p1 = np.asarray(x, dtype=np.float32) @ np.asarray(W1, dtype=np.float32)
    p1 = p1.astype(NP_BF16).reshape(NCORES, NGRP, GRP, 100, F1)
    return np.ascontiguousarray(p1.transpose(0, 1, 3, 2, 4)).reshape(
        NCORES, NGRP, 100, GRP * F1
    )


def _prep_cell(cell):
    cell = np.asarray(cell, dtype=np.float32)
    nrm = np.sqrt((cell * cell).sum(axis=1, keepdims=True))
    cv = cell / np.maximum(nrm, 1e-12)
    cv = cv.reshape(NCORES, GPC, 954)
    cv = np.ascontiguousarray(cv.transpose(0, 2, 1))  # [NCORES, 954, GPC]
    return cv.reshape(NCORES, 9, 106, GPC).astype(NP_BF16)


def _wchunk(w, kc):
    """[K, M] -> [kc, K//kc? ...] -> sbuf layout [kchunk_rows, nchunks, M]."""
    K, M = w.shape
    n = K // kc
    return np.ascontiguousarray(
        w.reshape(n, kc, M).transpose(1, 0, 2)
    )


def _bchunk(b, pc):
    """[F] -> [F//pc, pc]: column c holds chunk c of the bias, fp32."""
    return np.ascontiguousarray(b.reshape(pc, -1).T).astype(np.float32)


def _build_program():
    nc = bacc.Bacc("TRN2", target_bir_lowering=False, debug=False)

    def din(name, shape, dt=BF16):
        return nc.dram_tensor(name, list(shape), dt, kind="ExternalInput").ap()

    x1p = din("x1p", (NGRP, 100, GRP * F1))
    x2p = din("x2p", (NGRP, 100, GRP * F1))
    a1p = din("a1p", (NGRP, 100, GRP * 100))
    a2p = din("a2p", (NGRP, 100, GRP * 100))
    cellc = din("cellc", (9, 106, GPC))

    # contraction over features is zero-padded 78 -> 100: the PE streams at
    # ~0.44ns/col when the contraction dim is >= 96 vs ~0.85ns/col below 80
    wc2_d = din("wc2", (100, F2))
    wc3_d = din("wc3", (100, 2, F3))
    wg1_d = din("wg1", (104, 3, F2))
    wg2_d = din("wg2", (78, 2, 128))
    wr1_d = din("wr1", (106, 9, 512))
    wr2_d = din("wr2", (128, 4, 256))
    wr3_d = din("wr3", (128, 2, 128))
    wf1_d = din("wf1", (128, 3, 256))
    wf2_d = din("wf2", (128, 2, 128))
    wo_d = din("wo", (128, 2))

    bc1_d = din("bc1", (78, 1), F32)
    bc2_d = din("bc2", (78, 2), F32)
    bc3_d = din("bc3", (104, 3), F32)
    bg1_d = din("bg1", (78, 2), F32)
    bg2_d = din("bg2", (128, 1), F32)
    br1_d = din("br1", (128, 4), F32)
    br2_d = din("br2", (128, 2), F32)
    br3_d = din("br3", (128, 1), F32)
    bf1_d = din("bf1", (128, 2), F32)
    bf2_d = din("bf2", (128, 1), F32)
    bo_d = din("bo", (2, 1), F32)

    out_d = nc.dram_tensor("outT", [2, GPC], F32, kind="ExternalOutput").ap()

    with tile.TileContext(nc) as tc:
        from contextlib import ExitStack

        with ExitStack() as ctx:
            cpool = ctx.enter_context(tc.tile_pool(name="consts", bufs=1))

            def load(dram, shape, dt=BF16):
                nm = dram.name.split("_")[0]
                t = cpool.tile(list(shape), dt, tag=nm, name=nm)
                nc.sync.dma_start(t[:], dram[:])
                return t

            wc2 = load(wc2_d, (100, F2))
            wc3 = load(wc3_d, (100, 2, F3))
            wg1 = load(wg1_d, (104, 3, F2))
            wg2 = load(wg2_d, (78, 2, 128))
            wr1 = load(wr1_d, (106, 9, 512))
            wr2 = load(wr2_d, (128, 4, 256))
            wr3 = load(wr3_d, (128, 2, 128))
            wf1 = load(wf1_d, (128, 3, 256))
            wf2 = load(wf2_d, (128, 2, 128))
            wo = load(wo_d, (128, 2))
            bc1 = load(bc1_d, (78, 1), F32)
            bc2 = load(bc2_d, (78, 2), F32)
            bc3 = load(bc3_d, (104, 3), F32)
            bg1 = load(bg1_d, (78, 2), F32)
            bg2 = load(bg2_d, (128, 1), F32)
            br1 = load(br1_d, (128, 4), F32)
            br2 = load(br2_d, (128, 2), F32)
            br3 = load(br3_d, (128, 1), F32)
            bf1 = load(bf1_d, (128, 2), F32)
            bf2 = load(bf2_d, (128, 1), F32)
            bo = load(bo_d, (2, 1), F32)

            # persistent per-branch outputs
            pooled_raw = [
                [
                    cpool.tile([104, GPC], BF16, tag=f"pr{d}{c}", name=f"pr{d}{c}")
                    for c in range(3)
                ]
                for d in range(2)
            ]
            pooled = [
                [
                    cpool.tile([104, GPC], BF16, tag=f"pool{d}{c}", name=f"pool{d}{c}")
                    for c in range(3)
                ]
                for d in range(2)
            ]
            demb = [
                cpool.tile([128, GPC], BF16, tag=f"demb{d}", name=f"demb{d}")
                for d in range(2)
            ]
            c3T = cpool.tile([128, GPC], BF16, tag="c3T", name="c3T")

            # ---------------- cell branch (runs first: its DMAs prefetch
            # at t=0 and its long accumulation chains warm up the PE) ----
            with tc.tile_pool(name="cellp", bufs=1) as clp, tc.tile_pool(
                name="pscell", bufs=2, space=bass.MemorySpace.PSUM
            ) as cps:
                cell_sb = []
                for k in range(9):
                    t = clp.tile([106, GPC], BF16, tag=f"cell{k}", name=f"cell{k}")
                    nc.sync.dma_start(t[:], cellc[k])
                    cell_sb.append(t)
                c1 = clp.tile([128, 4 * GPC], BF16, tag="c1", name="c1")
                for m in range(4):
                    for n in range(2):
                        ps = cps.tile([128, 512], F32, tag="ps", name="ps")
                        for k in range(9):
                            nc.tensor.matmul(
                                ps[:],
                                wr1[:, k, m * 128 : (m + 1) * 128],
                                cell_sb[k][:, n * 512 : (n + 1) * 512],
                                start=(k == 0),
                                stop=(k == 8),
                            )
                        nc.scalar.activation(
                            c1[:, m * GPC + n * 512 : m * GPC + (n + 1) * 512],
                            ps[:],
                            RELU,
                            bias=br1[:, m : m + 1],
                        )
                c2 = clp.tile([128, 2 * GPC], BF16, tag="c2", name="c2")
                for m in range(2):
                    for n in range(2):
                        ps = cps.tile([128, 512], F32, tag="ps", name="ps")
                        for k in range(4):
                            nc.tensor.matmul(
                                ps[:],
                                wr2[:, k, m * 128 : (m + 1) * 128],
                                c1[:, k * GPC + n * 512 : k * GPC + (n + 1) * 512],
                                start=(k == 0),
                                stop=(k == 3),
                            )
                        nc.scalar.activation(
                            c2[:, m * GPC + n * 512 : m * GPC + (n + 1) * 512],
                            ps[:],
                            RELU,
                            bias=br2[:, m : m + 1],
                        )
                for n in range(2):
                    ps = cps.tile([128, 512], F32, tag="ps", name="ps")
                    for k in range(2):
                        nc.tensor.matmul(
                            ps[:],
                            wr3[:, k, :],
                            c2[:, k * GPC + n * 512 : k * GPC + (n + 1) * 512],
                            start=(k == 0),
                            stop=(k == 1),
                        )
                    nc.scalar.activation(
                        c3T[:, n * 512 : (n + 1) * 512], ps[:], IDENT, bias=br3[:]
                    )

            # ---------------- drug branches ----------------
            # Per group of 8 pairs: FOUR interleaved streams (2 drugs x 2
            # quads of 4 pairs) so the per-stream dependency chain
            # (agg1 -> act1 -> xw2 -> cast -> agg2 -> ...) is covered by
            # ~3 phases of other streams' PE work and the PE never idles
            # (keeps the tensor engine in the high DVFS p-state).
            # L3 uses the relu<->max swap: pool the raw aggregation from
            # PSUM, defer bias+relu to the pooled [104, GPC] tensor.
            with tc.tile_pool(name="io", bufs=6) as iop, tc.tile_pool(
                name="mid", bufs=8
            ) as midp, tc.tile_pool(name="p3p", bufs=16) as p3p, tc.tile_pool(
                name="psb", bufs=2, space=bass.MemorySpace.PSUM
            ) as psum:
                drug_io = ((x1p, a1p), (x2p, a2p))
                # zero the K-pad rows (78:100) of every rotating h tile once;
                # the in-loop activations only write rows 0:78
                # (memset starts at partition 64 -- DVE base-partition rule --
                # rows 64:78 are rewritten by the activations each iteration)
                for _ in range(8):
                    t1 = midp.tile([100, 400], BF16, tag="h1q", name="h1q")
                    nc.vector.memset(t1[64:100, :], 0.0)
                    t2 = midp.tile([100, 800], BF16, tag="h2q", name="h2q")
                    nc.vector.memset(t2[64:100, :], 0.0)
                for gi in range(NGRP):
                    tiles = []
                    for d, (xp, ap) in enumerate(drug_io):
                        p1t = iop.tile([100, GRP * F1], BF16, tag="p1t", name="p1t")
                        nc.sync.dma_start(p1t[:], xp[gi])
                        at = iop.tile([100, GRP * 100], BF16, tag="at", name="at")
                        nc.sync.dma_start(at[:], ap[gi])
                        tiles.append((p1t, at))

                    def make_quad(d, q):
                        p1t, at = tiles[d]
                        base = q * 4
                        st = {}

                        def pcols(j):
                            o = (base + j) * 100
                            return slice(o, o + 100)

                        def p2_agg1():
                            ph1 = psum.tile([78, 400], F32, tag="ph", name="ph1")
                            for j in range(4):
                                o = (base + j) * F1
                                nc.tensor.matmul(
                                    ph1[:, 100 * j : 100 * j + 100],
                                    p1t[:, o : o + F1],
                                    at[:, pcols(j)],
                                    start=True,
                                    stop=True,
                                )
                            h1q = midp.tile([100, 400], BF16, tag="h1q", name="h1q")
                            nc.scalar.activation(
                                h1q[0:78, :], ph1[:], RELU, bias=bc1[:]
                            )
                            st["h1q"] = h1q

                        def p3_xw2():
                            h1q = st["h1q"]
                            pp2 = [
                                psum.tile([100, 312], F32, tag="pp", name="pp2", bufs=4)
                                for _ in range(2)
                            ]
                            for j in range(4):
                                nc.tensor.matmul(
                                    pp2[j // 2][:, 156 * (j % 2) : 156 * (j % 2) + 156],
                                    h1q[:, 100 * j : 100 * j + 100],
                                    wc2[:],
                                    start=True,
                                    stop=True,
                                )
                            p2q = [
                                midp.tile([100, 312], BF16, tag="p2q", name="p2q")
                                for _ in range(2)
                            ]
                            nc.scalar.activation(
                                p2q[0][:, 0:156], pp2[0][:, 0:156], IDENT
                            )
                            nc.scalar.activation(
                                p2q[0][:, 156:312], pp2[0][:, 156:312], IDENT
                            )
                            nc.vector.tensor_copy(
                                p2q[1][:, 0:156], pp2[1][:, 0:156]
                            )
                            nc.vector.tensor_copy(
                                p2q[1][:, 156:312], pp2[1][:, 156:312]
                            )
                            st["p2q"] = p2q

                        def p4_agg2():
                            p2q = st["p2q"]
                            ph2 = [
                                psum.tile([78, 400], F32, tag="ph", name="ph2")
                                for _ in range(2)
                            ]
                            for c in range(2):
                                for j in range(4):
                                    o = 156 * (j % 2) + 78 * c
                                    nc.tensor.matmul(
                                        ph2[c][:, 100 * j : 100 * j + 100],
                                        p2q[j // 2][:, o : o + 78],
                                        at[:, pcols(j)],
                                        start=True,
                                        stop=True,
                                    )
                            h2q = midp.tile([100, 800], BF16, tag="h2q", name="h2q")
                            for c in range(2):
                                nc.scalar.activation(
                                    h2q[0:78, 400 * c : 400 * c + 400],
                                    ph2[c][:],
                                    RELU,
                                    bias=bc2[:, c : c + 1],
                                )
                            st["h2q"] = h2q

                        def p5_xw3():
                            h2q = st["h2q"]
                            p3l = []
                            for j in range(4):
                                pp3 = psum.tile(
                                    [100, 312], F32, tag="pp", name="pp3", bufs=4
                                )
                                nc.tensor.matmul(
                                    pp3[:],
                                    h2q[:, 100 * j : 100 * j + 100],
                                    wc3[:, 0, :],
                                    start=True,
                                    stop=False,
                                )
                                nc.tensor.matmul(
                                    pp3[:],
                                    h2q[:, 400 + 100 * j : 400 + 100 * j + 100],
                                    wc3[:, 1, :],
                                    start=False,
                                    stop=True,
                                )
                                p3 = p3p.tile([100, 312], BF16, tag="p3", name="p3")
                                # chunk-0 half first: agg3's first chunk only
                                # needs cols 0:156 of every pair's p3
                                if j % 2 == 0:
                                    nc.scalar.activation(
                                        p3[:, 0:156], pp3[:, 0:156], IDENT
                                    )
                                    nc.scalar.activation(
                                        p3[:, 156:312], pp3[:, 156:312], IDENT
                                    )
                                else:
                                    nc.vector.tensor_copy(
                                        p3[:, 0:156], pp3[:, 0:156]
                                    )
                                    nc.vector.tensor_copy(
                                        p3[:, 156:312], pp3[:, 156:312]
                                    )
                                p3l.append(p3)
                            st["p3l"] = p3l

                        def p6_agg3():
                            p3l = st["p3l"]
                            goff = 2 * (gi * GRP + base)
                            for c in range(3):
                                ph3 = psum.tile(
                                    [104, 8, 50], F32, tag="ph3", name="ph3"
                                )
                                for j in range(4):
                                    nc.tensor.matmul(
                                        ph3[:, 2 * j : 2 * j + 2, :],
                                        p3l[j][:, 104 * c : 104 * c + 104],
                                        at[:, pcols(j)],
                                        start=True,
                                        stop=True,
                                    )
                                nc.vector.tensor_reduce(
                                    pooled_raw[d][c][:, goff : goff + 8],
                                    ph3[:],
                                    AXX,
                                    MAXOP,
                                )

                        return (p2_agg1, p3_xw2, p4_agg2, p5_xw3, p6_agg3)

                    streams = [make_quad(d, q) for d in range(2) for q in range(2)]
                    for phase_fns in zip(*streams):
                        for fn in phase_fns:
                            fn()

            # ---------------- drug FC heads ----------------
            with tc.tile_pool(name="fc", bufs=1) as pool, tc.tile_pool(
                name="psfc", bufs=2, space=bass.MemorySpace.PSUM
            ) as psum:
                # deferred bias+relu of the max-pooled GCN outputs
                for d in range(2):
                    for c in range(3):
                        nc.scalar.activation(
                            pooled[d][c][:],
                            pooled_raw[d][c][:],
                            RELU,
                            bias=bc3[:, c : c + 1],
                        )
                for d in range(2):
                    gfc = pool.tile([78, 2 * GPC], BF16, tag=f"gfc{d}", name=f"gfc{d}")
                    for m in range(2):
                        for n in range(2):
                            ps = psum.tile([78, 512], F32, tag="ps", name="ps")
                            for k in range(3):
                                nc.tensor.matmul(
                                    ps[:],
                                    wg1[:, k, m * 78 : (m + 1) * 78],
                                    pooled[d][k][:, n * 512 : (n + 1) * 512],
                                    start=(k == 0),
                                    stop=(k == 2),
                                )
                            nc.scalar.activation(
                                gfc[:, m * GPC + n * 512 : m * GPC + (n + 1) * 512],
                                ps[:],
                                RELU,
                                bias=bg1[:, m : m + 1],
                            )
                    for n in range(2):
                        ps = psum.tile([128, 512], F32, tag="ps", name="ps")
                        for k in range(2):
                            nc.tensor.matmul(
                                ps[:],
                                wg2[:, k, :],
                                gfc[:, k * GPC + n * 512 : k * GPC + (n + 1) * 512],
                                start=(k == 0),
                                stop=(k == 1),
                            )
                        nc.scalar.activation(
                            demb[d][:, n * 512 : (n + 1) * 512],
                            ps[:],
                            IDENT,
                            bias=bg2[:],
                        )

                # ---------------- head ----------------
                xcs = [demb[0], demb[1], c3T]
                hf1 = pool.tile([128, 2 * GPC], BF16, tag="hf1", name="hf1")
                for m in range(2):
                    for n in range(2):
                        ps = psum.tile([128, 512], F32, tag="ps", name="ps")
                        for k in range(3):
                            nc.tensor.matmul(
                                ps[:],
                                wf1[:, k, m * 128 : (m + 1) * 128],
                                xcs[k][:, n * 512 : (n + 1) * 512],
                                start=(k == 0),
                                stop=(k == 2),
                            )
                        nc.scalar.activation(
                            hf1[:, m * GPC + n * 512 : m * GPC + (n + 1) * 512],
                            ps[:],
                            RELU,
                            bias=bf1[:, m : m + 1],
                        )
                hf2 = pool.tile([128, GPC], BF16, tag="hf2", name="hf2")
                for n in range(2):
                    ps = psum.tile([128, 512], F32, tag="ps", name="ps")
                    for k in range(2):
                        nc.tensor.matmul(
                            ps[:],
                            wf2[:, k, :],
                            hf1[:, k * GPC + n * 512 : k * GPC + (n + 1) * 512],
                            start=(k == 0),
                            stop=(k == 1),
                        )
                    nc.scalar.activation(
                        hf2[:, n * 512 : (n + 1) * 512], ps[:], RELU, bias=bf2[:]
                    )
                osb = pool.tile([2, GPC], F32, tag="osb", name="osb")
                for n in range(2):
                    ps = psum.tile([2, 512], F32, tag="ps", name="ps")
                    nc.tensor.matmul(
                        ps[:],
                        wo[:],
                        hf2[:, n * 512 : (n + 1) * 512],
                        start=True,
                        stop=True,
                    )
                    nc.scalar.activation(
                        osb[:, n * 512 : (n + 1) * 512], ps[:], IDENT, bias=bo[:]
                    )
                nc.sync.dma_start(out_d[:], osb[:])

    nc.compile()
    return nc


def kernel(x1, edge_index1, batch1, x2, edge_index2, batch2, cell,
           Wc1, bc1, Wc2, bc2, Wc3, bc3, Wg1, bg1, Wg2, bg2,
           Wr1, br1, Wr2, br2, Wr3, br3, Wf1, bf1, Wf2, bf2, Wo, bo):
    if "nc" not in _CACHE:
        _CACHE["nc"] = _build_program()
    nc = _CACHE["nc"]

    x1p = _tile_p1(x1, Wc1)
    x2p = _tile_p1(x2, Wc1)
    a1p = _build_at_pairs(edge_index1)
    a2p = _build_at_pairs(edge_index2)
    cellc = _prep_cell(cell)

    bf = lambda a: np.asarray(a, dtype=np.float32).astype(NP_BF16)
    f32 = lambda a: np.asarray(a, dtype=np.float32)

    def padk(a, k=100):
        out = np.zeros((k,) + a.shape[1:], dtype=a.dtype)
        out[: a.shape[0]] = a
        return out

    shared = dict(
        wc2=padk(bf(Wc2)),
        wc3=padk(bf(_wchunk(np.asarray(Wc3, np.float32), 78))),
        wg1=bf(_wchunk(np.asarray(Wg1, np.float32), 104)),
        wg2=bf(_wchunk(np.asarray(Wg2, np.float32), 78)),
        wr1=bf(_wchunk(np.asarray(Wr1, np.float32), 106)),
        wr2=bf(_wchunk(np.asarray(Wr2, np.float32), 128)),
        wr3=bf(_wchunk(np.asarray(Wr3, np.float32), 128)),
        wf1=bf(_wchunk(np.asarray(Wf1, np.float32), 128)),
        wf2=bf(_wchunk(np.asarray(Wf2, np.float32), 128)),
        wo=bf(Wo),
        bc1=f32(bc1).reshape(78, 1),
        bc2=_bchunk(f32(bc2), 2),
        bc3=_bchunk(f32(bc3), 3),
        bg1=_bchunk(f32(bg1), 2),
        bg2=f32(bg2).reshape(128, 1),
        br1=_bchunk(f32(br1), 4),
        br2=_bchunk(f32(br2), 2),
        br3=f32(br3).reshape(128, 1),
        bf1=_bchunk(f32(bf1), 2),
        bf2=f32(bf2).reshape(128, 1),
        bo=f32(bo).reshape(2, 1),
    )

    in_maps = []
    for c in range(NCORES):
        m = dict(shared)
        m["x1p"] = x1p[c]
        m["x2p"] = x2p[c]
        m["a1p"] = a1p[c]
        m["a2p"] = a2p[c]
        m["cellc"] = cellc[c]
        in_maps.append(m)

    res = run_bass_kernel_spmd(nc, in_maps, list(range(NCORES)))
    _CACHE["last_result"] = res
    out = np.concatenate(
        [np.asarray(res.results[c]["outT"], np.float32).T for c in range(NCORES)],
        axis=0,
    )
    return out
